# revision 9
# baseline (speedup 1.0000x reference)
"""Trainium2 Bass kernel for nn_AffinityHead — 1-descriptor-per-edge design.

Math: out[e] = w2 . relu(z1[src[e]] + z2[dst[e]]) + b2 with per-node tables
    z1[n] = x_n@W1a - c_n@W1c        (w2 magnitude/sign folded via channel
    z2[n] = x_n@W1b + c_n@W1c + b1    permutation + scaling, host-side)

SWDGE descriptor generation (~2.7ns/desc, 4 queues saturated) is the HW
bottleneck; the baseline spent 2 descriptors per edge (gather z1[src] AND
z2[dst]). This kernel spends ONE:

  dst side: SWDGE dma_gather of z2[dst] rows from a DRAM table (as before).
  src side: NO descriptors. Edges are host-sorted by (dst_half, src_block
    =src//128); each 128-node block's z1 rows live in SBUF (built by a
    phase-Z pass), and a PE matmul with a host-shipped one-hot lhsT
    "expands" z1[src] for each slot range: psum[slot, c] = sum_node
    oh[node, slot] * z1blk[node, c].

Pipeline: z2-pass (PE) -> z2 DRAM table (p-major permuted rows => 2KB-per-
partition write descriptors; host permutes gather indices to match) ->
dst-gathers start per half; z1-pass + expansion matmuls woven on PE behind
the gathers; DVE adds psum+g2, ACT relu, DVE range-reduces -> per-edge out.
"""

import numpy as np
import ml_dtypes

N, C, E = 50000, 128, 800000
N_CORES = 8
EC = E // N_CORES
HALF = 32768
MT = 2048
N_PAD = 50176            # 392*128; last MT macro tile is half-size
TILE_T = 1024
NQ = 4

_cache = {}


def _zperm(n):
    """Table row for node n: p-major within its MT macro tile."""
    m = (n // MT) * MT
    r = n - m
    ng = np.minimum(N_PAD - m, MT) // 128
    return m + (r % 128) * ng + r // 128


def _prep_host(tokens, coords, edge_index, w1, b1, w2, b2):
    tokens = np.asarray(tokens, dtype=np.float32)[0]
    coords = np.asarray(coords, dtype=np.float32)[0]
    ei = np.asarray(edge_index).astype(np.int64)
    w1 = np.asarray(w1, dtype=np.float32)
    b1 = np.asarray(b1, dtype=np.float32)
    w2 = np.asarray(w2, dtype=np.float32)
    b2 = np.asarray(b2, dtype=np.float32)

    w2v = w2[:, 0]
    order = np.argsort(w2v < 0, kind="stable")
    p_pos = int((w2v >= 0).sum())
    scale = np.abs(w2v[order])
    w1p = w1[:, order] * scale[None, :]
    b1p = b1[order] * scale

    W1a, W1b = w1p[:C], w1p[C:2 * C]
    W1cx, W1cy = w1p[2 * C], w1p[2 * C + 1]

    bf = ml_dtypes.bfloat16
    tokT_np = np.zeros((C, N_PAD), dtype=np.float32)
    tokT_np[:, :N] = tokens.T
    cooT_np = np.zeros((4, N_PAD), dtype=np.float32)
    cooT_np[0, :N] = coords[:, 0]
    cooT_np[1, :N] = coords[:, 1]
    cooT_np[2, :] = 1.0
    w1ab_np = np.concatenate([W1a, W1b], axis=1)         # [C, 2C]
    w1c3_np = np.zeros((4, 2 * C), dtype=np.float32)
    w1c3_np[0] = np.concatenate([-W1cx, W1cx])
    w1c3_np[1] = np.concatenate([-W1cy, W1cy])
    w1c3_np[2] = np.concatenate([np.zeros(C, np.float32), b1p])
    tokT_np = tokT_np.astype(bf)
    cooT_np = cooT_np.astype(bf)
    w1ab_np = w1ab_np.astype(bf)
    w1c3_np = w1c3_np.astype(bf)

    src_all, dst_all = ei[0], ei[1]
    NB = (N + 127) // 128            # 391 src blocks

    # SPMD: ONE program for all cores -> slot layout must be IDENTICAL.
    # The edge->core assignment is ours: deal each (half, block) bucket's
    # edges round-robin across cores so per-core counts differ by <=1; the
    # shared run length is ceil(total/8), then 32-aligned (PE tile_position
    # col offsets are 32-quantized).
    h_all = (dst_all >= HALF).astype(np.int64)
    b_all = src_all // 128
    key_all = h_all * NB + b_all
    order_all = np.argsort(key_all, kind="stable")   # bucket-grouped edges
    kk_sorted = key_all[order_all]
    cnt = np.bincount(key_all, minlength=2 * NB)     # per-bucket totals
    first = np.zeros(2 * NB, dtype=np.int64)
    np.cumsum(cnt[:-1], out=first[1:])
    rank_all = np.arange(E) - first[kk_sorted]       # rank within bucket
    core_of = rank_all % N_CORES                     # deal to cores
    rank_in_core = rank_all // N_CORES

    rmax = -(-(-(-cnt // N_CORES)) // 32) * 32       # ceil/8 then 32-align
    rmax = rmax.reshape(2, NB)
    sc_h = []
    for h in range(2):
        tot = int(rmax[h].sum())
        pad = (-tot) % TILE_T
        rmax[h, NB - 1] += pad
        sc_h.append(tot + pad)
    SC0, SC1 = int(sc_h[0]), int(sc_h[1])
    SC = SC0 + SC1
    NTT = SC // TILE_T
    in_maps = []
    pos_maps = []

    # slot offsets of each (half, block) run
    run_off = np.zeros((2, NB), dtype=np.int64)
    off = 0
    for h in range(2):
        for b in range(NB):
            run_off[h, b] = off
            off += int(rmax[h, b])
    assert off == SC

    # compile-time pieces: (half, block, s0, s1) split at 128-slot groups,
    # then split for tile_position legality: col offset o with M slots needs
    # o==0 (any M), o==64 (M<=64), o in {32,96} (M<=32)
    pieces = []
    for h in range(2):
        for b in range(NB):
            s0, s1 = int(run_off[h, b]), int(run_off[h, b] + rmax[h, b])
            p = s0
            while p < s1:
                q = min(s1, (p // 128 + 1) * 128)
                o = p % 128
                if o == 32 and q - p > 32:
                    q = p + 32
                pieces.append((h, b, p, q))
                p = q

    dperm_all = _zperm(dst_all)
    for c in range(N_CORES):
        sel = core_of == c
        eidx = order_all[sel]                       # global edge ids, bucket order
        kk = kk_sorted[sel]
        slot = run_off.reshape(-1)[kk] + rank_in_core[sel]

        dstl = np.zeros(SC, dtype=np.int16)
        dstl[slot] = (dperm_all[eidx] - h_all[eidx] * HALF).astype(np.int16)
        oh = np.zeros((128, SC), dtype=ml_dtypes.bfloat16)
        oh[src_all[eidx] % 128, slot] = np.float32(1.0)
        pm = np.full(SC, -1, dtype=np.int64)
        pm[slot] = eidx                              # GLOBAL edge index

        dw = np.tile(np.ascontiguousarray(dstl.reshape(-1, 16).T), (8, 1))
        in_maps.append({
            "tokT": tokT_np, "cooT": cooT_np, "w1ab": w1ab_np,
            "w1c3": w1c3_np, "oh": np.ascontiguousarray(oh),
            "didx": np.ascontiguousarray(dw),
        })
        pos_maps.append(pm)

    meta = (SC0, SC1, tuple(pieces))
    return meta, p_pos, in_maps, pos_maps, float(b2[0])


def _build(meta, p_pos):
    import concourse.bacc as bacc
    import concourse.mybir as mybir
    import concourse.tile as tile

    SC0, SC1, pieces = meta
    SC = SC0 + SC1
    NTT = SC // TILE_T
    bf = mybir.dt.bfloat16
    NB = (N + 127) // 128
    NCH = N_PAD // (8 * 128)         # 49 z1 chunks of 8 blocks

    nc = bacc.Bacc("TRN2", target_bir_lowering=False, debug=False,
                   num_devices=N_CORES, num_swdge_queues=NQ)

    tokT = nc.dram_tensor("tokT", [C, N_PAD], bf, kind="ExternalInput").ap()
    cooT = nc.dram_tensor("cooT", [4, N_PAD], bf, kind="ExternalInput").ap()
    w1ab = nc.dram_tensor("w1ab", [C, 2 * C], bf, kind="ExternalInput").ap()
    w1c3 = nc.dram_tensor("w1c3", [4, 2 * C], bf, kind="ExternalInput").ap()
    ohd = nc.dram_tensor("oh", [128, SC], bf, kind="ExternalInput").ap()
    didx = nc.dram_tensor("didx", [128, SC // 16], mybir.dt.int16,
                          kind="ExternalInput").ap()
    outd = nc.dram_tensor("out", [128, SC // 128], mybir.dt.float32,
                          kind="ExternalOutput").ap()
    z2_dram = nc.dram_tensor("z2tbl", [N_PAD, C], bf).ap()

    # expansion pieces grouped by 512-slot psum window
    NW = SC // 512
    win_pieces = [[] for _ in range(NW)]
    win_maxblk = [0] * NW
    for (h, b, s0, s1) in pieces:
        w = s0 // 512
        win_pieces[w].append((b, s0, s1))
        win_maxblk[w] = max(win_maxblk[w], b)
    # monotone prefix requirement for z1 chunks (chunk = 8 blocks)
    need_chunk = [0] * NW
    run_max = 0
    for w in range(NW):
        run_max = max(run_max, win_maxblk[w])
        need_chunk[w] = run_max // 8 + 1   # chunks 0..need-1 must be emitted

    with tile.TileContext(nc) as tc:
        with (
            tc.tile_pool(name="wpool", bufs=1) as wpool,
            tc.tile_pool(name="zpsum", bufs=2, space="PSUM") as zpsum,
            tc.tile_pool(name="epsum", bufs=4, space="PSUM") as epsum,
            tc.tile_pool(name="ztok", bufs=2) as ztok,
            tc.tile_pool(name="zcoo", bufs=2) as zcoo,
            tc.tile_pool(name="zstage", bufs=2) as zstage,
            tc.tile_pool(name="z1pool", bufs=1) as z1pool,
            tc.tile_pool(name="ipool", bufs=1) as ipool,
            tc.tile_pool(name="ohpool", bufs=4) as ohpool,
            tc.tile_pool(name="gpool", bufs=12) as gpool,
            tc.tile_pool(name="spool", bufs=4) as spool,
            tc.tile_pool(name="rpool", bufs=4) as rpool,
            tc.tile_pool(name="opool", bufs=4) as opool,
            tc.tile_pool(name="obuf", bufs=1) as obuf,
        ):
            didx_sb = ipool.tile([128, SC // 16], mybir.dt.int16)
            nc.sync.dma_start(out=didx_sb[:], in_=didx[:])
            outbuf = obuf.tile([128, SC // 128], mybir.dt.float32)

            w1ab_sb = wpool.tile([C, 2 * C], bf)
            nc.sync.dma_start(out=w1ab_sb[:], in_=w1ab[:])
            w1c3_sb = wpool.tile([128, 2 * C], bf)
            nc.vector.memset(w1c3_sb[:], 0.0)
            nc.sync.dma_start(out=w1c3_sb[0:4, :], in_=w1c3[:])

            # ---------------- phase Z2: build z2 DRAM table ----------------
            for mi, m in enumerate(range(0, N_PAD, MT)):
                mw = min(MT, N_PAD - m)
                ng = mw // 128
                tok_mt = ztok.tile([C, MT], bf, tag="tokA")
                nc.scalar.dma_start(out=tok_mt[:, :mw], in_=tokT[:, m:m + mw])
                # coo lhsT zero-padded to K=128 (uniform PE config, no
                # tile-size switches); zeros persist across buffer reuse
                coo_mt = zcoo.tile([128, MT], bf, tag="cooA")
                if mi < 2:
                    nc.vector.memset(coo_mt[:], 0.0)
                nc.scalar.dma_start(out=coo_mt[0:4, :mw], in_=cooT[:, m:m + mw])
                zv = z2_dram[m:m + mw, :].rearrange("(p g) c -> p g c", p=128)
                for ch in range(ng // 8):
                    ps = zpsum.tile([128, 8, C], mybir.dt.float32, tag="zp")
                    for g in range(8):
                        cc = (ch * 8 + g) * 128
                        nc.tensor.matmul(ps[:, g, :],
                                         lhsT=tok_mt[:, cc:cc + 128],
                                         rhs=w1ab_sb[:, C:2 * C],
                                         start=True, stop=False)
                        nc.tensor.matmul(ps[:, g, :],
                                         lhsT=coo_mt[:, cc:cc + 128],
                                         rhs=w1c3_sb[:, C:2 * C],
                                         start=False, stop=True)
                    zs = zstage.tile([128, 8, C], bf, tag="zs")
                    nc.scalar.copy(out=zs[:], in_=ps[:])
                    nc.scalar.dma_start(out=zv[:, ch * 8:(ch + 1) * 8, :],
                                        in_=zs[:])

            # ---------- gathers (GpSimd stream, ordered h0 then h1) --------
            gtiles = []
            qn = 0
            for t in range(NTT):
                lo = 0 if (t * TILE_T) < SC0 else HALF
                hi = HALF if lo == 0 else N_PAD
                g2 = gpool.tile([128, 8, C], bf, tag="g2")
                nc.gpsimd.dma_gather(
                    out_ap=g2[:], in_ap=z2_dram[lo:hi, :],
                    idxs_ap=didx_sb[:, t * (TILE_T // 16):(t + 1) * (TILE_T // 16)],
                    num_idxs=TILE_T, num_idxs_reg=TILE_T,
                    elem_size=C, elem_step=C, queue_num=qn % NQ)
                qn += 1
                gtiles.append(g2)

            # ------- z1 pass + expansion + tail, woven by dependency -------
            z1chunks = []

            def emit_z1_chunk(k):
                # chunk k = blocks 8k..8k+7 = nodes 1024k..1024k+1024
                m0 = k * 1024
                tok_mt = ztok.tile([C, 1024], bf, tag="tokB")
                nc.scalar.dma_start(out=tok_mt[:], in_=tokT[:, m0:m0 + 1024])
                coo_mt = zcoo.tile([128, 1024], bf, tag="cooB")
                if k < 2:
                    nc.vector.memset(coo_mt[:], 0.0)
                nc.scalar.dma_start(out=coo_mt[0:4, :], in_=cooT[:, m0:m0 + 1024])
                ps = zpsum.tile([128, 8, C], mybir.dt.float32, tag="zp")
                for g in range(8):
                    nc.tensor.matmul(ps[:, g, :],
                                     lhsT=tok_mt[:, g * 128:g * 128 + 128],
                                     rhs=w1ab_sb[:, 0:C],
                                     start=True, stop=False)
                    nc.tensor.matmul(ps[:, g, :],
                                     lhsT=coo_mt[:, g * 128:g * 128 + 128],
                                     rhs=w1c3_sb[:, 0:C],
                                     start=False, stop=True)
                z1c = z1pool.tile([128, 8, C], bf, tag=f"z1c{k}")
                nc.scalar.copy(out=z1c[:], in_=ps[:])
                z1chunks.append(z1c)

            emitted_chunks = 0
            for t in range(NTT):
                # expansion psum windows for this tile
                pss = []
                oh_t = ohpool.tile([128, TILE_T], bf, tag="oh")
                nc.sync.dma_start(out=oh_t[:],
                                  in_=ohd[:, t * TILE_T:(t + 1) * TILE_T])
                for wi in (2 * t, 2 * t + 1):
                    while emitted_chunks < min(need_chunk[wi], NCH):
                        emit_z1_chunk(emitted_chunks)
                        emitted_chunks += 1
                    ps = epsum.tile([128, 4, C], mybir.dt.float32, tag="ep")
                    for (b, s0, s1) in win_pieces[wi]:
                        k, gs = b // 8, b % 8
                        p0 = s0 % 128
                        g = (s0 - wi * 512) // 128
                        # explicit tile_position: auto path rejects
                        # base partition 96 (legal here since M<=32)
                        nc.tensor.matmul(
                            ps[p0:p0 + (s1 - s0), g, :],
                            lhsT=oh_t[:, s0 - t * TILE_T:s1 - t * TILE_T],
                            rhs=z1chunks[k][:, gs, :],
                            start=True, stop=True,
                            tile_position=(0, p0))
                    pss.append(ps)

                g2 = gtiles[t]
                s = spool.tile([128, 8, C], bf, tag="s")
                nc.vector.tensor_add(s[:, 0:4, :], g2[:, 0:4, :], pss[0][:])
                nc.vector.tensor_add(s[:, 4:8, :], g2[:, 4:8, :], pss[1][:])
                r = rpool.tile([128, 8, C], bf, tag="r")
                nc.scalar.activation(r[:], s[:],
                                     mybir.ActivationFunctionType.Relu)
                o_pos = opool.tile([128, 8], mybir.dt.float32, tag="op")
                o_neg = opool.tile([128, 8], mybir.dt.float32, tag="on")
                if p_pos > 0:
                    nc.vector.reduce_sum(o_pos[:], r[:, :, 0:p_pos],
                                         axis=mybir.AxisListType.X)
                else:
                    nc.vector.memset(o_pos[:], 0.0)
                if p_pos < C:
                    nc.vector.reduce_sum(o_neg[:], r[:, :, p_pos:C],
                                         axis=mybir.AxisListType.X)
                else:
                    nc.vector.memset(o_neg[:], 0.0)
                nc.vector.tensor_sub(outbuf[:, t * 8:(t + 1) * 8],
                                     o_pos[:], o_neg[:])

            nc.sync.dma_start(out=outd[:], in_=outbuf[:])

    nc.compile()
    return nc


def kernel(tokens, coords, edge_index, w1, b1, w2, b2):
    from concourse.bass_utils import run_bass_kernel_spmd

    meta, p_pos, in_maps, pos_maps, b2v = _prep_host(
        tokens, coords, edge_index, w1, b1, w2, b2)

    key = (meta, p_pos)
    if key not in _cache:
        _cache[key] = _build(meta, p_pos)
    nc = _cache[key]

    last_err = None
    for _attempt in range(3):
        try:
            res = run_bass_kernel_spmd(nc, in_maps, list(range(N_CORES)))
            break
        except Exception as e:
            last_err = e
            import time as _time
            _time.sleep(20)
    else:
        raise last_err

    SC0, SC1, _ = meta
    SC = SC0 + SC1
    NTT = SC // TILE_T
    out = np.empty(E, dtype=np.float32)
    for c in range(N_CORES):
        o = res.results[c]["out"]                # [128, SC/128]
        r = o.reshape(128, NTT, 8).transpose(1, 2, 0).reshape(-1)
        pm = pos_maps[c]
        valid = pm >= 0
        out[pm[valid]] = r[valid]
    out += b2v
    return out.reshape(1, E, 1)
